# revision 7
# baseline (speedup 1.0000x reference)
"""Trainium2 Bass kernel for masked cosine-similarity attention scores.

Problem: nn_MultiHeadedAttention_2 (sparse_attention, memory-bound)
  query [16, 1, 1024] f32, key [16, 8192, 1024] f32, mask [16, 8192] int32
  out   [16, 16, 8192] f32 = relu(cos_sim_per_head(q, k) masked) / Lk

Math (per batch b, head h, key position l):
  num[h,l] = sum_d q[h,d] * k[l, h*64+d]
  kn[h,l]  = ||k[l, h*64:(h+1)*64]||
  p        = relu(num / (qn[h] * kn)) * mask[l] / Lk
           = relu(sum_d qtilde[h,d] * k[...]) * exp(-0.5*ln(kn^2) + lnm[l])
  where qtilde = q / (qn * Lk) is folded on the host (input prep) and
  lnm[l] = 0 if mask else -1e30 (exp(...-1e30) == 0 -> exact masked zero).
  The reference's EPS=1e-8 guard on qn*kn is unreachable for randn inputs
  (qn, kn ~ sqrt(64)), so it is not emulated.

Sharding: data-parallel over batch B=16 -> 2 batches per core x 8 cores.

Scheduling discipline: walrus permits only ONE semaphore wait per compute
instruction, so every op is arranged to have at most one un-elided
cross-engine dependency (constants staged through DVE, non-rotating
ppair buffer, PSUM drain on DVE so the PE transpose's deps merge).

Self-contained: only imports the platform libs from /opt/trn_rl_repo.
"""

import os
import sys

sys.path.insert(0, "/opt/trn_rl_repo")

import numpy as np

import concourse.bass as bass
import concourse.mybir as mybir
from concourse.tile import TileContext
from concourse.bass_utils import run_bass_kernel_spmd

# Keep the number of active DMA completion-sem lanes low: the kernel-tail
# Drain waits on every active proc's semaphore and walrus rejects
# instructions with too many sync waits. Lanes are bookkeeping sems (FIFO
# per ring), not HW queues, so this does not serialize the transfers.
import concourse.tile_sem_assignment as _tsa

_tsa.NUM_HWDGE_SEMS = 2
_tsa.NUM_SWDGE_GLOBAL_SEMS = 2

# The walrus build in this environment accepts at most ONE sync wait per
# instruction. Tile's scheduler can emit several (cross-engine RAW + WAR +
# DMA-lane waits). Splitting the extra waits into standalone EventSemaphore
# instructions on the same engine is semantically identical: the engine's
# sequencer executes them in order immediately before the instruction.
import orjson as _orjson


def _split_multi_waits(bir_bytes: bytes) -> bytes:
    m = _orjson.loads(bir_bytes)
    changed = False
    for fn in m.get("functions", []):
        for bb in fn.get("blocks", []):
            insts = bb.get("instructions")
            if not insts:
                continue
            out_list = []
            for inst in insts:
                si = inst.get("sync_info")
                waits = (si or {}).get("on_wait") or []
                if len(waits) > 1:
                    changed = True
                    for k, w in enumerate(waits[:-1]):
                        out_list.append(
                            {
                                "debug": inst.get("debug", 0),
                                "engine": inst["engine"],
                                "ins": [],
                                "name": f"{inst['name']}_wsplit{k}",
                                "opcode": "EventSemaphore",
                                "outs": [],
                                "sync_info": {"on_update": [], "on_wait": [w]},
                            }
                        )
                    si["on_wait"] = [waits[-1]]
                out_list.append(inst)
            bb["instructions"] = out_list
    return _orjson.dumps(m) if changed else bir_bytes


_orig_to_json_bytes = bass.Bass.to_json_bytes


def _patched_to_json_bytes(self, *a, **kw):
    return _split_multi_waits(_orig_to_json_bytes(self, *a, **kw))


bass.Bass.to_json_bytes = _patched_to_json_bytes

F32 = mybir.dt.float32
BF16 = mybir.dt.bfloat16
I32 = mybir.dt.int32
Alu = mybir.AluOpType
Act = mybir.ActivationFunctionType
AX = mybir.AxisListType

H = 16      # heads
DK = 64     # head dim
DM = 1024   # d_model
P = 128     # SBUF partitions
N_CORES = 8

# Compute dtype for the streamed key data ("f32" or "bf16").
PRECISION = os.environ.get("COSSIM_PRECISION", "bf16")


def build_nc(n_batch: int, lk: int, precision: str = PRECISION) -> bass.Bass:
    """Build the per-core Bass program.

    Per-core DRAM I/O:
      key   [n_batch, lk, 1024] f32   (shard of the key tensor)
      qb    [n_batch, 128, 1024] cdt  (host-broadcast qtilde rows)
      maskr [n_batch, 128, lk/128] i32 (mask with l split as l = t*128 + p)
      out   [n_batch, 16, lk] f32
    """
    assert n_batch == 2, "kernel assumes a batch pair per core"
    cdt = BF16 if precision == "bf16" else F32
    ntiles = lk // P            # 128-key subtiles per batch
    TG = 4                      # subtiles per DMA group
    ngroups = ntiles // TG

    nc = bass.Bass()
    key_in = nc.declare_dram_parameter("key", [n_batch, lk, DM], F32, isOutput=False)
    qb_in = nc.declare_dram_parameter("qb", [n_batch, P, DM], cdt, isOutput=False)
    mask_in = nc.declare_dram_parameter(
        "maskr", [n_batch, P, ntiles], I32, isOutput=False
    )
    ident_in = nc.declare_dram_parameter("ident", [P, P], F32, isOutput=False)
    out = nc.declare_dram_parameter("out", [n_batch, H, lk], F32, isOutput=True)
    out_flat = out.rearrange("b h l -> (b h) l")  # [32, lk]

    with TileContext(nc) as tc:
        with (
            tc.tile_pool(name="const", bufs=1) as cpool,
            tc.tile_pool(name="kbig", bufs=2) as kpool,
            tc.tile_pool(name="work", bufs=3) as wpool,
            tc.tile_pool(name="small", bufs=6) as spool,
            tc.tile_pool(name="outp", bufs=1) as opool,
            tc.tile_pool(name="psum", bufs=4, space="PSUM") as pspool,
        ):
            # constants, staged through DVE so consumers only dep on DVE
            ident_r = cpool.tile([P, P], F32, name="ident_r")
            nc.gpsimd.dma_start(out=ident_r[:], in_=ident_in[:])
            ident = cpool.tile([P, P], F32, name="ident_s")
            nc.vector.tensor_copy(ident[:], ident_r[:])

            qbs, lnms = [], []
            for b in range(n_batch):
                qb_r = cpool.tile([P, DM], cdt, name=f"qbr{b}")
                nc.gpsimd.dma_start(out=qb_r[:], in_=qb_in[b])
                qb = cpool.tile([P, DM], cdt, name=f"qbs{b}")
                nc.vector.tensor_copy(qb[:], qb_r[:])
                qbs.append(qb)
                maskt = cpool.tile([P, ntiles], I32, name=f"maskt{b}")
                nc.gpsimd.dma_start(out=maskt[:], in_=mask_in[b])
                maskf = cpool.tile([P, ntiles], F32, name=f"maskf{b}")
                nc.vector.tensor_copy(maskf[:], maskt[:])
                lnm = cpool.tile([P, ntiles], F32, name=f"lnm{b}")
                # lnm = (mask - 1) * 1e30  ->  {0 -> -1e30, 1 -> 0}
                nc.vector.tensor_scalar(
                    lnm[:], maskf[:], -1.0, 1.0e30, Alu.add, Alu.mult
                )
                lnms.append(lnm)

            outacc = opool.tile([2 * H, lk], F32, name="outacc")
            # non-rotating staging buffer for pre-transpose results
            ppair = opool.tile([P, 2 * H * ntiles], F32, name="ppair")

            pending = None  # deferred PSUM drain (tp tile, l0)
            for g in range(ngroups):
                kts = []
                for b in range(n_batch):
                    kt = kpool.tile([P, TG * DM], cdt, name="kt", tag=f"kt{b}")
                    src = key_in[b].rearrange("(t p) c -> p t c", p=P)[
                        :, g * TG : (g + 1) * TG, :
                    ]
                    dst = kt.rearrange("p (t c) -> p t c", c=DM)
                    if cdt == F32:
                        nc.sync.dma_start(out=dst, in_=src)
                    else:
                        nc.gpsimd.dma_start(out=dst, in_=src)  # casts f32->bf16
                    kts.append(kt)
                for j in range(TG):
                    t = g * TG + j
                    l0 = t * P
                    pp = ppair[:, t * 2 * H : (t + 1) * 2 * H]
                    for b in range(n_batch):
                        ks = kts[b][:, j * DM : (j + 1) * DM]
                        prod = wpool.tile([P, DM], cdt, name="prod", tag="prod")
                        nc.vector.tensor_tensor(prod[:], ks, qbs[b][:], Alu.mult)
                        # stage through DVE so kt has a single reader proc
                        # (walrus allows only one WAR wait on the kt DMA)
                        kss = wpool.tile([P, DM], cdt, name="kss", tag="kss")
                        nc.vector.tensor_copy(kss[:], ks)
                        sq = wpool.tile([P, DM], cdt, name="sq", tag="sq")
                        nc.scalar.activation(sq[:], kss[:], Act.Square)
                        num = spool.tile([P, H], F32, name="num", tag="num")
                        nc.vector.reduce_sum(
                            num[:], prod.rearrange("p (h d) -> p h d", d=DK), axis=AX.X
                        )
                        s2 = spool.tile([P, H], F32, name="s2", tag="s2")
                        nc.vector.reduce_sum(
                            s2[:], sq.rearrange("p (h d) -> p h d", d=DK), axis=AX.X
                        )
                        lns = spool.tile([P, H], F32, name="lns", tag="lns")
                        nc.scalar.activation(lns[:], s2[:], Act.Ln)
                        rk = spool.tile([P, H], F32, name="rk", tag="rk")
                        nc.scalar.activation(
                            rk[:],
                            lns[:],
                            Act.Exp,
                            bias=lnms[b][:, t : t + 1],
                            scale=-0.5,
                        )
                        nr = spool.tile([P, H], F32, name="nr", tag="nr")
                        nc.scalar.activation(nr[:], num[:], Act.Relu)
                        # pp[:, b*16:(b+1)*16] = relu(num) * rk; both inputs
                        # are ACT products so the TT carries one merged wait
                        nc.vector.tensor_tensor(
                            pp[:, b * H : (b + 1) * H], nr[:], rk[:], Alu.mult
                        )
                    tp = pspool.tile([2 * H, P], F32, name="tp", tag="tp")
                    nc.tensor.transpose(tp[:], pp, ident[:])
                    if pending is not None:
                        ptp, pl0 = pending
                        nc.vector.tensor_copy(outacc[:, pl0 : pl0 + P], ptp[:])
                    pending = (tp, l0)
            ptp, pl0 = pending
            nc.vector.tensor_copy(outacc[:, pl0 : pl0 + P], ptp[:])

            nc.sync.dma_start(out=out_flat, in_=outacc[:])
    return nc


_NC_CACHE: dict = {}


def _get_nc(n_batch, lk, precision=PRECISION):
    key = (n_batch, lk, precision)
    if key not in _NC_CACHE:
        _NC_CACHE[key] = build_nc(n_batch, lk, precision)
    return _NC_CACHE[key]


def prep_inputs(query, key, mask, n_cores=N_CORES, precision=PRECISION):
    """Shard + host-side input prep (layout & folding of scalars into qtilde)."""
    B, lk, dm = key.shape
    assert dm == DM
    nb = B // n_cores
    cdt_np = mybir.dt.np(BF16 if precision == "bf16" else F32)

    q = query.reshape(B, H, DK).astype(np.float64)
    qn = np.sqrt((q * q).sum(-1))  # [B, H]
    qt = q / (qn[:, :, None] * float(lk))  # qtilde [B, H, DK]
    qb = np.ascontiguousarray(
        np.broadcast_to(qt.reshape(B, 1, DM), (B, P, DM))
    ).astype(cdt_np)

    ntiles = lk // P
    maskr = np.ascontiguousarray(
        mask.reshape(B, ntiles, P).transpose(0, 2, 1)
    ).astype(np.int32)

    ident = np.eye(P, dtype=np.float32)

    in_maps = []
    for c in range(n_cores):
        sl = slice(c * nb, (c + 1) * nb)
        in_maps.append(
            {
                "key": np.ascontiguousarray(key[sl]),
                "qb": qb[sl],
                "maskr": maskr[sl],
                "ident": ident,
            }
        )
    return in_maps


def kernel(query, key, mask, trace=False):
    B, lk, _ = key.shape
    nb = B // N_CORES
    nc = _get_nc(nb, lk)
    in_maps = prep_inputs(query, key, mask)
    res = run_bass_kernel_spmd(nc, in_maps, list(range(N_CORES)), trace=trace)
    outs = [res.results[i]["out"] for i in range(N_CORES)]
    full = np.concatenate(outs, axis=0)  # [B, H, lk]
    if trace:
        kernel.last_exec_time_ns = res.exec_time_ns
        kernel.last_result = res
    return full


if __name__ == "__main__":
    # smoke test at reduced size
    rng = np.random.default_rng(0)
    B, lk = 16, 1024
    query = rng.standard_normal((B, 1, DM), dtype=np.float32)
    key = rng.standard_normal((B, lk, DM), dtype=np.float32)
    mask = rng.integers(0, 2, (B, lk)).astype(np.int32)
    out = kernel(query, key, mask)
    print("out", out.shape, out.dtype, float(np.abs(out).max()))


# revision 12
# speedup vs baseline: 554.8067x; 554.8067x over previous
"""Trainium2 Bass kernel for masked cosine-similarity attention scores.

Problem: nn_MultiHeadedAttention_2 (sparse_attention, memory-bound)
  query [16, 1, 1024] f32, key [16, 8192, 1024] f32, mask [16, 8192] int32
  out   [16, 16, 8192] f32 = relu(cos_sim_per_head(q, k) masked) / Lk

Math (per batch b, head h, key position l):
  num[h,l] = sum_d q[h,d] * k[l, h*64+d]
  kn[h,l]  = ||k[l, h*64:(h+1)*64]||
  p        = relu(num / (qn[h] * kn)) * mask[l] / Lk
           = relu(sum_d qtilde[h,d] * k[...]) * exp(-0.5*ln(kn^2) + lnm[l])
  where qtilde = q / (qn * Lk) is folded on the host (input prep) and
  lnm[l] = 0 if mask else -1e30 (exp(...-1e30) == 0 -> exact masked zero).
  The reference's EPS=1e-8 guard on qn*kn is unreachable for randn inputs
  (qn, kn ~ sqrt(64)), so it is not emulated.

Sharding: data-parallel over batch B=16 -> 2 batches per core x 8 cores.

Scheduling discipline: walrus permits only ONE semaphore wait per compute
instruction, so every op is arranged to have at most one un-elided
cross-engine dependency (constants staged through DVE, non-rotating
ppair buffer, PSUM drain on DVE so the PE transpose's deps merge).

Self-contained: only imports the platform libs from /opt/trn_rl_repo.
"""

import os
import sys

sys.path.insert(0, "/opt/trn_rl_repo")

import numpy as np

import concourse.bass as bass
import concourse.mybir as mybir
from concourse.tile import TileContext
from concourse.bass_utils import run_bass_kernel_spmd

# Keep the number of active DMA completion-sem lanes low: the kernel-tail
# Drain waits on every active proc's semaphore and walrus rejects
# instructions with too many sync waits. Lanes are bookkeeping sems (FIFO
# per ring), not HW queues, so this does not serialize the transfers.
import concourse.tile_sem_assignment as _tsa

_tsa.NUM_HWDGE_SEMS = 2
_tsa.NUM_SWDGE_GLOBAL_SEMS = 2

# The walrus build in this environment accepts at most ONE sync wait per
# instruction. Tile's scheduler can emit several (cross-engine RAW + WAR +
# DMA-lane waits). Splitting the extra waits into standalone EventSemaphore
# instructions on the same engine is semantically identical: the engine's
# sequencer executes them in order immediately before the instruction.
import orjson as _orjson


def _split_multi_waits(bir_bytes: bytes) -> bytes:
    m = _orjson.loads(bir_bytes)
    changed = False
    for fn in m.get("functions", []):
        for bb in fn.get("blocks", []):
            insts = bb.get("instructions")
            if not insts:
                continue
            out_list = []
            for inst in insts:
                si = inst.get("sync_info")
                waits = (si or {}).get("on_wait") or []
                if len(waits) > 1:
                    changed = True
                    for k, w in enumerate(waits[:-1]):
                        out_list.append(
                            {
                                "debug": inst.get("debug", 0),
                                "engine": inst["engine"],
                                "ins": [],
                                "name": f"{inst['name']}_wsplit{k}",
                                "opcode": "EventSemaphore",
                                "outs": [],
                                "sync_info": {"on_update": [], "on_wait": [w]},
                            }
                        )
                    si["on_wait"] = [waits[-1]]
                out_list.append(inst)
            bb["instructions"] = out_list
    return _orjson.dumps(m) if changed else bir_bytes


_orig_to_json_bytes = bass.Bass.to_json_bytes


def _patched_to_json_bytes(self, *a, **kw):
    return _split_multi_waits(_orig_to_json_bytes(self, *a, **kw))


bass.Bass.to_json_bytes = _patched_to_json_bytes

F32 = mybir.dt.float32
BF16 = mybir.dt.bfloat16
I32 = mybir.dt.int32
Alu = mybir.AluOpType
Act = mybir.ActivationFunctionType
AX = mybir.AxisListType

H = 16      # heads
DK = 64     # head dim
DM = 1024   # d_model
P = 128     # SBUF partitions
N_CORES = 8

# Compute dtype for the streamed key data ("f32" or "bf16").
PRECISION = os.environ.get("COSSIM_PRECISION", "bf16")


def build_nc(n_batch: int, lk: int, precision: str = PRECISION) -> bass.Bass:
    """Build the per-core Bass program.

    Per-core DRAM I/O:
      key   [n_batch, lk, 1024] f32   (shard of the key tensor)
      qb    [n_batch, 128, 1024] cdt  (host-broadcast qtilde rows)
      maskr [n_batch, 128, lk/128] i32 (mask with l split as l = t*128 + p)
      out   [n_batch, 16, lk] f32
    """
    assert n_batch == 2, "kernel assumes a batch pair per core"
    cdt = BF16 if precision == "bf16" else F32
    ntiles = lk // P            # 128-key subtiles per batch
    TG = 4                      # subtiles per DMA group
    ngroups = ntiles // TG

    nc = bass.Bass()
    key_in = nc.declare_dram_parameter("key", [n_batch, lk, DM], F32, isOutput=False)
    qb_in = nc.declare_dram_parameter("qb", [n_batch, P, DM], cdt, isOutput=False)
    mask_in = nc.declare_dram_parameter(
        "maskr", [n_batch, P, ntiles], I32, isOutput=False
    )
    ident_in = nc.declare_dram_parameter("ident", [P, P], F32, isOutput=False)
    out = nc.declare_dram_parameter("out", [n_batch, H, lk], F32, isOutput=True)
    out_flat = out.rearrange("b h l -> (b h) l")  # [32, lk]

    with TileContext(nc) as tc:
        with (
            tc.tile_pool(name="const", bufs=1) as cpool,
            tc.tile_pool(name="kbig", bufs=2) as kpool,
            tc.tile_pool(name="work", bufs=3) as wpool,
            tc.tile_pool(name="small", bufs=6) as spool,
            tc.tile_pool(name="outp", bufs=1) as opool,
            tc.tile_pool(name="psum", bufs=4, space="PSUM") as pspool,
        ):
            # constants, staged through DVE so consumers only dep on DVE
            ident_r = cpool.tile([P, P], F32, name="ident_r")
            nc.gpsimd.dma_start(out=ident_r[:], in_=ident_in[:])
            ident = cpool.tile([P, P], F32, name="ident_s")
            nc.vector.tensor_copy(ident[:], ident_r[:])

            qbs, lnms = [], []
            for b in range(n_batch):
                qb_r = cpool.tile([P, DM], cdt, name=f"qbr{b}")
                nc.gpsimd.dma_start(out=qb_r[:], in_=qb_in[b])
                qb = cpool.tile([P, DM], cdt, name=f"qbs{b}")
                nc.vector.tensor_copy(qb[:], qb_r[:])
                qbs.append(qb)
                maskt = cpool.tile([P, ntiles], I32, name=f"maskt{b}")
                nc.gpsimd.dma_start(out=maskt[:], in_=mask_in[b])
                maskf = cpool.tile([P, ntiles], F32, name=f"maskf{b}")
                nc.vector.tensor_copy(maskf[:], maskt[:])
                lnm = cpool.tile([P, ntiles], F32, name=f"lnm{b}")
                # lnm = (mask - 1) * 1e30  ->  {0 -> -1e30, 1 -> 0}
                nc.vector.tensor_scalar(
                    lnm[:], maskf[:], -1.0, 1.0e30, Alu.add, Alu.mult
                )
                lnms.append(lnm)

            outacc = opool.tile([2 * H, lk], F32, name="outacc")
            # non-rotating staging buffer for pre-transpose results
            ppair = opool.tile([P, 2 * H * ntiles], F32, name="ppair")

            pending = None  # deferred PSUM drain (tp tile, l0)
            for g in range(ngroups):
                kts = []
                for b in range(n_batch):
                    kt = kpool.tile([P, TG * DM], cdt, name="kt", tag=f"kt{b}")
                    src = key_in[b].rearrange("(t p) c -> p t c", p=P)[
                        :, g * TG : (g + 1) * TG, :
                    ]
                    dst = kt.rearrange("p (t c) -> p t c", c=DM)
                    if cdt == F32:
                        nc.sync.dma_start(out=dst, in_=src)
                    else:
                        nc.gpsimd.dma_start(out=dst, in_=src)  # casts f32->bf16
                    kts.append(kt)
                for j in range(TG):
                    t = g * TG + j
                    l0 = t * P
                    pp = ppair[:, t * 2 * H : (t + 1) * 2 * H]
                    for b in range(n_batch):
                        ks = kts[b][:, j * DM : (j + 1) * DM]
                        prod = wpool.tile([P, DM], cdt, name="prod", tag="prod")
                        nc.vector.tensor_tensor(prod[:], ks, qbs[b][:], Alu.mult)
                        sq = wpool.tile([P, DM], cdt, name="sq", tag="sq")
                        nc.scalar.activation(sq[:], ks, Act.Square)

                        def seg_reduce(dst, src):
                            # per-head sum over d: [P, 16, 64] -> [P, 16]
                            if cdt == BF16:
                                # first level as a bf16 TT add (2x mode),
                                # then one f32-out reduce on half the data
                                half = wpool.tile(
                                    [P, DM // 2], cdt, name="half", tag="half"
                                )
                                s3 = src.rearrange("p (h d) -> p h d", d=DK)
                                nc.vector.tensor_tensor(
                                    half.rearrange("p (h d) -> p h d", d=DK // 2),
                                    s3[:, :, 0 : DK // 2],
                                    s3[:, :, DK // 2 : DK],
                                    Alu.add,
                                )
                                red_in = half.rearrange(
                                    "p (h d) -> p h d", d=DK // 2
                                )
                            else:
                                red_in = src.rearrange("p (h d) -> p h d", d=DK)
                            nc.vector.reduce_sum(dst[:], red_in, axis=AX.X)

                        num = spool.tile([P, H], F32, name="num", tag="num")
                        seg_reduce(num, prod)
                        s2 = spool.tile([P, H], F32, name="s2", tag="s2")
                        seg_reduce(s2, sq)
                        lns = spool.tile([P, H], F32, name="lns", tag="lns")
                        nc.scalar.activation(lns[:], s2[:], Act.Ln)
                        rk = spool.tile([P, H], F32, name="rk", tag="rk")
                        nc.scalar.activation(
                            rk[:],
                            lns[:],
                            Act.Exp,
                            bias=lnms[b][:, t : t + 1],
                            scale=-0.5,
                        )
                        # pp[:, b*16:(b+1)*16] = max(num, 0) * rk
                        nc.vector.scalar_tensor_tensor(
                            pp[:, b * H : (b + 1) * H],
                            num[:],
                            0.0,
                            rk[:],
                            Alu.max,
                            Alu.mult,
                        )
                    tp = pspool.tile([2 * H, P], F32, name="tp", tag="tp")
                    nc.tensor.transpose(tp[:], pp, ident[:])
                    if pending is not None:
                        ptp, pl0 = pending
                        nc.vector.tensor_copy(outacc[:, pl0 : pl0 + P], ptp[:])
                    pending = (tp, l0)
            ptp, pl0 = pending
            nc.vector.tensor_copy(outacc[:, pl0 : pl0 + P], ptp[:])

            nc.sync.dma_start(out=out_flat, in_=outacc[:])
    return nc


_NC_CACHE: dict = {}


def _get_nc(n_batch, lk, precision=PRECISION):
    key = (n_batch, lk, precision)
    if key not in _NC_CACHE:
        _NC_CACHE[key] = build_nc(n_batch, lk, precision)
    return _NC_CACHE[key]


def prep_inputs(query, key, mask, n_cores=N_CORES, precision=PRECISION):
    """Shard + host-side input prep (layout & folding of scalars into qtilde)."""
    B, lk, dm = key.shape
    assert dm == DM
    nb = B // n_cores
    cdt_np = mybir.dt.np(BF16 if precision == "bf16" else F32)

    q = query.reshape(B, H, DK).astype(np.float64)
    qn = np.sqrt((q * q).sum(-1))  # [B, H]
    qt = q / (qn[:, :, None] * float(lk))  # qtilde [B, H, DK]
    qb = np.ascontiguousarray(
        np.broadcast_to(qt.reshape(B, 1, DM), (B, P, DM))
    ).astype(cdt_np)

    ntiles = lk // P
    maskr = np.ascontiguousarray(
        mask.reshape(B, ntiles, P).transpose(0, 2, 1)
    ).astype(np.int32)

    ident = np.eye(P, dtype=np.float32)

    in_maps = []
    for c in range(n_cores):
        sl = slice(c * nb, (c + 1) * nb)
        in_maps.append(
            {
                "key": np.ascontiguousarray(key[sl]),
                "qb": qb[sl],
                "maskr": maskr[sl],
                "ident": ident,
            }
        )
    return in_maps


class _Runner:
    """Cached PJRT executable for one built Bass program.

    Mirrors bass2jax.run_bass_via_pjrt but jits ONCE, and feeds the
    global (unsharded) arrays directly: shard_map splits axis 0 across
    the 8 cores, which is exactly the per-core batch shard.
    """

    def __init__(self, nc, n_cores):
        import jax
        from jax.sharding import Mesh, PartitionSpec
        from jax.experimental.shard_map import shard_map
        from concourse import bass2jax as b2j

        b2j.install_neuronx_cc_hook()
        self.jax = jax
        self.n_cores = n_cores
        part_name = (
            nc.partition_id_tensor.name if nc.partition_id_tensor else None
        )
        in_names, out_names, out_avals, zero_outs = [], [], [], []
        for alloc in nc.m.functions[0].allocations:
            if not isinstance(alloc, mybir.MemoryLocationSet):
                continue
            name = alloc.memorylocations[0].name
            if alloc.kind == "ExternalInput":
                if name != part_name:
                    in_names.append(name)
            elif alloc.kind == "ExternalOutput":
                out_names.append(name)
                shape = tuple(alloc.tensor_shape)
                dtype = mybir.dt.np(alloc.dtype)
                out_avals.append(jax.core.ShapedArray(shape, dtype))
                zero_outs.append(np.zeros(shape, dtype))
        self.in_names, self.out_names = in_names, out_names
        self.out_avals, self.zero_outs = out_avals, zero_outs
        n_params, n_outs = len(in_names), len(out_names)

        bind_in_names = in_names + out_names
        if part_name is not None:
            bind_in_names = bind_in_names + [part_name]

        def _body(*args):
            operands = list(args)
            if part_name is not None:
                operands.append(b2j.partition_id_tensor())
            outs = b2j._bass_exec_p.bind(
                *operands,
                out_avals=tuple(out_avals),
                in_names=tuple(bind_in_names),
                out_names=tuple(out_names),
                lowering_input_output_aliases=(),
                sim_require_finite=True,
                sim_require_nnan=True,
                nc=nc,
            )
            return tuple(outs)

        devices = jax.devices()[:n_cores]
        self.mesh = Mesh(np.asarray(devices), ("core",))
        in_specs = (PartitionSpec("core"),) * (n_params + n_outs)
        out_specs = (PartitionSpec("core"),) * n_outs
        self.fn = jax.jit(
            shard_map(
                _body,
                mesh=self.mesh,
                in_specs=in_specs,
                out_specs=out_specs,
                check_rep=False,
            ),
            donate_argnums=tuple(range(n_params, n_params + n_outs)),
            keep_unused=True,
        )

    def global_args(self, global_ins: dict):
        args = [global_ins[name] for name in self.in_names]
        args += [
            np.zeros((self.n_cores * z.shape[0], *z.shape[1:]), z.dtype)
            for z in self.zero_outs
        ]
        return args

    def __call__(self, global_ins: dict):
        out_arrs = self.fn(*self.global_args(global_ins))
        return {
            name: np.asarray(out_arrs[i]) for i, name in enumerate(self.out_names)
        }


_RUNNER_CACHE: dict = {}


def _get_runner(n_batch, lk, precision=PRECISION):
    key = (n_batch, lk, precision)
    if key not in _RUNNER_CACHE:
        nc = _get_nc(n_batch, lk, precision)
        if not nc.is_finalized():
            nc.finalize()
        _RUNNER_CACHE[key] = _Runner(nc, N_CORES)
    return _RUNNER_CACHE[key]


def global_inputs(query, key, mask, precision=PRECISION):
    """Host prep producing the UNSHARDED arrays fed to shard_map (axis 0
    splits evenly across the 8 cores == batch sharding). Zero-copy for key."""
    B, lk, dm = key.shape
    assert dm == DM
    cdt_np = mybir.dt.np(BF16 if precision == "bf16" else F32)

    q = query.reshape(B, H, DK).astype(np.float64)
    qn = np.sqrt((q * q).sum(-1))  # [B, H]
    qt = q / (qn[:, :, None] * float(lk))  # qtilde [B, H, DK]
    qb = np.ascontiguousarray(
        np.broadcast_to(qt.reshape(B, 1, DM), (B, P, DM))
    ).astype(cdt_np)

    ntiles = lk // P
    maskr = np.ascontiguousarray(
        mask.reshape(B, ntiles, P).transpose(0, 2, 1)
    ).astype(np.int32)

    ident = np.tile(np.eye(P, dtype=np.float32), (N_CORES, 1)).reshape(
        N_CORES * P, P
    )
    return {"key": np.ascontiguousarray(key), "qb": qb, "maskr": maskr,
            "ident": ident}


def kernel(query, key, mask, trace=False):
    B, lk, _ = key.shape
    nb = B // N_CORES
    runner = _get_runner(nb, lk)
    gins = global_inputs(query, key, mask)
    out = runner(gins)["out"]  # [B*?, H, lk] concat over cores on axis 0
    full = out.reshape(B, H, lk)
    return full


if __name__ == "__main__":
    # smoke test at reduced size
    rng = np.random.default_rng(0)
    B, lk = 16, 1024
    query = rng.standard_normal((B, 1, DM), dtype=np.float32)
    key = rng.standard_normal((B, lk, DM), dtype=np.float32)
    mask = rng.integers(0, 2, (B, lk)).astype(np.int32)
    out = kernel(query, key, mask)
    print("out", out.shape, out.dtype, float(np.abs(out).max()))
